# revision 7
# baseline (speedup 1.0000x reference)
"""Expert-parallel batched SwiGLU FFN for Trainium2 (8 NeuronCores, Bass/Tile).

Problem: out[e] = (silu(x[e] @ w1[e].T + b1[e]) * (x[e] @ w3[e].T + b3[e])) @ w2[e].T + b2[e]
with E=8, T=512, D_IN=7168, D_INT=2048, fp32.

Sharding: expert-parallel — core e owns expert e end-to-end, no communication.

Layout strategy: the TensorEngine contracts over the SBUF partition dim, so both
matmul operands need the contraction dim (d, then f) on partitions. DRAM-side we
stage every tensor pre-transposed on the host (free host-side rearrangement during
sharding, exactly how MoE frameworks store weights), so every device DMA is a
natural contiguous load and the kernel does zero on-chip transposes.

Numerics: matmuls run in float32r (fp32 storage, FP22 multiply, fp32 PSUM
accumulate) — full bf16-rate on the PE with ~1e-4 relative error.

Per-core schedule:
  phase 1 (h1t/h3t = w.T-contractions over d, output f-major):
    d-chunk outer loop (4 chunks x 14 subtiles) with SBUF partial accumulation
    so only one x-chunk is SBUF-resident at a time; per chunk, stream
    [128,14,256] weight blocks of w1t/w3t, 14-deep PSUM accumulation groups,
    DVE eviction into persistent partials. Last chunk fuses bias + silu + mul
    into gt (fp32r, in-place over the h3 partial).
  phase 2: out = gt.T @ w2t + b2, d-blocks of 512, 16-deep PSUM accumulation,
    DVE bias-add eviction, 1MB output DMAs.
"""

import numpy as np

import concourse.bacc as bacc
import concourse.mybir as mybir
import concourse.tile as tile
from concourse.bass_utils import run_bass_kernel_spmd

# Problem shape (hardcoded per contest contract).
E = 8
T = 512
D = 7168
F = 2048
P = 128

DO = D // P  # 56 d-subtiles
FO = F // P  # 16 f-subtiles
TT = T // P  # 4 t-subtiles

CH = 14  # d-subtiles per phase-1 chunk
NCH = DO // CH  # 4 chunks
FBW = 256  # phase-1 weight block width (f)
NFB = F // FBW  # 8 blocks
DBW = 512  # phase-2 block width (d)
NDB = D // DBW  # 14 blocks

F32 = mybir.dt.float32
F32R = mybir.dt.float32r

_NC = None


def _emit(nc, tc, xt, w1t, w3t, w2t, b1, b3, b2r, out):
    add = mybir.AluOpType.add
    mult = mybir.AluOpType.mult
    silu = mybir.ActivationFunctionType.Silu

    xt_r = xt.ap().rearrange("(o p) t -> p o t", p=P)  # [P, DO, T]
    w1t_r = w1t.ap().rearrange("(o p) f -> p o f", p=P)  # [P, DO, F]
    w3t_r = w3t.ap().rearrange("(o p) f -> p o f", p=P)  # [P, DO, F]
    w2t_r = w2t.ap().rearrange("(o p) d -> p o d", p=P)  # [P, FO, D]
    b1_r = b1.ap().rearrange("(o p) -> p o", p=P)  # [P, FO]
    b3_r = b3.ap().rearrange("(o p) -> p o", p=P)  # [P, FO]
    out_r = out.ap().rearrange("(o p) d -> p o d", p=P)  # [P, TT, D]

    HCH = CH // 2  # 7: half-chunk (separate tiles => fine-grained DMA->MM deps)
    FBW1 = 512  # phase-1 weight block width (f)
    NFB1 = F // FBW1  # 4
    NFL = FBW1 // P  # 4 f-subtiles per block
    HFO = FO // 2  # 8: phase-2 w2 half block

    with tile.TileContext(nc) as tc:
        with (
            tc.tile_pool(name="persist", bufs=1) as persist,
            tc.tile_pool(name="wsp", bufs=5) as wsp,  # shared weight stream pool
        ):
            # h3 partial, finally overwritten in-place with gt (fp32r).
            h3p = persist.tile([P, FO, T], F32R, tag="h3p")
            b1s = persist.tile([P, FO], F32, tag="b1s")
            b3s = persist.tile([P, FO], F32, tag="b3s")
            def dma_wblk_half(w_r, c, fb, half, eng):
                """One [P, HCH, FBW1] half-block of a phase-1 weight tile."""
                wb = wsp.tile([P, HCH, FBW1], F32R, tag="wblk")
                lo = c * CH + half * HCH
                eng.dma_start(
                    wb[:], w_r[:, lo : lo + HCH, fb * FBW1 : (fb + 1) * FBW1]
                )
                return wb

            def dma_w2b_half(db, half, eng):
                """One [P, HFO, DBW] half of a phase-2 w2 block."""
                sl = slice(db * DBW, (db + 1) * DBW)
                w2h = wsp.tile([P, HFO, DBW], F32R, tag="wblk")
                fsl = slice(0, HFO) if half == 0 else slice(HFO, FO)
                eng.dma_start(w2h[:], w2t_r[:, fsl, sl])
                return w2h

            def dma_w2b(db):
                return (
                    dma_w2b_half(db, 0, nc.sync),
                    dma_w2b_half(db, 1, nc.scalar),
                )

            w2_prefetch = []

            # ---------------- phase 1 ----------------
            with (
                tc.tile_pool(name="p1", bufs=1) as p1,
                tc.tile_pool(name="xtp", bufs=3) as xtp,
                tc.tile_pool(name="s1p", bufs=2) as s1p,
                tc.tile_pool(name="ps1", bufs=8, space="PSUM") as psum1,
            ):
                h1p = p1.tile([P, FO, T], F32, tag="h1p")

                def dma_xt_half(c, half, eng):
                    xh = xtp.tile([P, HCH, T], F32R, tag="xt")
                    lo = c * CH + half * HCH
                    eng.dma_start(xh[:], xt_r[:, lo : lo + HCH, :])
                    return xh

                # chunk 0 / block 0: the first psum group needs only
                # xtA + wbA(w1); land those first, one per queue, so the PE
                # starts ~15us in. Their counterpart halves queue right behind.
                xt0a = dma_xt_half(0, 0, nc.sync)
                wb00a = dma_wblk_half(w1t_r, 0, 0, 0, nc.scalar)
                wb00b = dma_wblk_half(w1t_r, 0, 0, 1, nc.sync)
                xt0b = dma_xt_half(0, 1, nc.scalar)
                xt_next = (xt0a, xt0b)
                for c in range(NCH):
                    xt_ab = xt_next
                    for fb in range(NFB1):
                        if c == 1 and fb == 0:
                            nc.sync.dma_start(b1s[:], b1_r)
                            nc.sync.dma_start(b3s[:], b3_r)
                        if c + 1 < NCH:
                            # spread next chunk's xt, one half per queue
                            if fb == 1:
                                xa = dma_xt_half(c + 1, 0, nc.sync)
                            elif fb == 2:
                                xt_next = (xa, dma_xt_half(c + 1, 1, nc.scalar))
                        s1_tiles = {}
                        for wi, w_r in ((0, w1t_r), (1, w3t_r)):
                            if c == 0 and fb == 0 and wi == 0:
                                wba, wbb = wb00a, wb00b
                            else:
                                wba = dma_wblk_half(w_r, c, fb, 0, nc.sync)
                                wbb = dma_wblk_half(w_r, c, fb, 1, nc.scalar)
                            for fl in range(NFL):
                                ft = fb * NFL + fl
                                ps = psum1.tile([P, T], F32, tag="ps")
                                for o in range(CH):
                                    half, ol = divmod(o, HCH)
                                    wb = wba if half == 0 else wbb
                                    xh = xt_ab[half]
                                    nc.tensor.matmul(
                                        ps[:],
                                        wb[:, ol, fl * P : (fl + 1) * P],
                                        xh[:, ol, :],
                                        start=(o == 0),
                                        stop=(o == CH - 1),
                                    )
                                hp = h1p if wi == 0 else h3p
                                if c == 0:
                                    nc.vector.tensor_copy(hp[:, ft, :], ps[:])
                                elif c < NCH - 1:
                                    nc.vector.tensor_add(
                                        hp[:, ft, :], hp[:, ft, :], ps[:]
                                    )
                                elif wi == 0:
                                    # h1 complete: s1 = silu(h1 + b1)
                                    nc.vector.tensor_add(
                                        hp[:, ft, :], hp[:, ft, :], ps[:]
                                    )
                                    s1_t = s1p.tile([P, T], F32, tag="s1")
                                    nc.scalar.activation(
                                        s1_t[:],
                                        hp[:, ft, :],
                                        silu,
                                        bias=b1s[:, ft : ft + 1],
                                    )
                                    s1_tiles[fl] = s1_t
                                else:
                                    # h3 complete: gt = (h3 + b3) * s1, fp32r,
                                    # written in place over the h3 partial.
                                    nc.vector.tensor_add(
                                        hp[:, ft, :], hp[:, ft, :], ps[:]
                                    )
                                    nc.vector.scalar_tensor_tensor(
                                        out=hp[:, ft, :],
                                        in0=hp[:, ft, :],
                                        scalar=b3s[:, ft : ft + 1],
                                        in1=s1_tiles[fl][:],
                                        op0=add,
                                        op1=mult,
                                    )
                        if c == NCH - 1 and fb == 2:
                            # phase-2 head start: db0 A-half early
                            w2_pre_a0 = dma_w2b_half(0, 0, nc.sync)
                    if c == NCH - 1:
                        # rest of the phase-2 head start
                        w2_prefetch.append(
                            (w2_pre_a0, dma_w2b_half(0, 1, nc.scalar))
                        )
                        w2_prefetch.append(
                            (
                                dma_w2b_half(1, 0, nc.sync),
                                dma_w2b_half(1, 1, nc.scalar),
                            )
                        )

            gt = h3p  # [P, FO, T] fp32r

            # ---------------- phase 2 ----------------
            with (
                tc.tile_pool(name="b2p", bufs=2) as b2p,
                tc.tile_pool(name="osp", bufs=2) as osp,
                tc.tile_pool(name="ps2", bufs=8, space="PSUM") as psum2,
            ):
                for db in range(NDB):
                    if db < len(w2_prefetch):
                        w2a, w2b_ = w2_prefetch[db]
                    else:
                        w2a, w2b_ = dma_w2b(db)
                    b2sl = b2p.tile([P, DBW], F32, tag="b2sl")
                    nc.scalar.dma_start(
                        b2sl[:], b2r.ap()[:, db * DBW : (db + 1) * DBW]
                    )
                    ost = osp.tile([P, TT, DBW], F32, tag="ost")
                    for tt in range(TT):
                        ps = psum2.tile([P, DBW], F32, tag="ps2")
                        for fo in range(FO):
                            w2h = w2a if fo < HFO else w2b_
                            nc.tensor.matmul(
                                ps[:],
                                gt[:, fo, tt * P : (tt + 1) * P],
                                w2h[:, fo % HFO, :],
                                start=(fo == 0),
                                stop=(fo == FO - 1),
                            )
                        nc.vector.tensor_add(
                            ost[:, tt, :],
                            ps[:],
                            b2sl[:],
                        )
                        if db == NDB - 1:
                            # stream the final block out per t-subtile
                            eng = nc.sync if tt % 2 == 0 else nc.scalar
                            eng.dma_start(
                                out_r[:, tt, db * DBW : (db + 1) * DBW],
                                ost[:, tt, :],
                            )
                    if db < NDB - 1:
                        dsl = slice(db * DBW, (db + 1) * DBW)
                        nc.sync.dma_start(out_r[:, :2, dsl], ost[:, :2, :])
                        nc.scalar.dma_start(out_r[:, 2:, dsl], ost[:, 2:, :])


def build():
    global _NC
    if _NC is not None:
        return _NC
    nc = bacc.Bacc("TRN2", target_bir_lowering=False, debug=False, num_devices=E)
    xt = nc.dram_tensor("xt", [D, T], F32R, kind="ExternalInput")
    w1t = nc.dram_tensor("w1t", [D, F], F32R, kind="ExternalInput")
    w3t = nc.dram_tensor("w3t", [D, F], F32R, kind="ExternalInput")
    w2t = nc.dram_tensor("w2t", [F, D], F32R, kind="ExternalInput")
    b1 = nc.dram_tensor("b1", [F], F32, kind="ExternalInput")
    b3 = nc.dram_tensor("b3", [F], F32, kind="ExternalInput")
    b2r = nc.dram_tensor("b2r", [P, D], F32, kind="ExternalInput")
    out = nc.dram_tensor("out", [T, D], F32, kind="ExternalOutput")
    _emit(nc, None, xt, w1t, w3t, w2t, b1, b3, b2r, out)
    nc.compile()
    _NC = nc
    return nc


def make_in_maps(x, w1, b1, w3, b3, w2, b2):
    x = np.asarray(x, dtype=np.float32)
    w1 = np.asarray(w1, dtype=np.float32)
    b1 = np.asarray(b1, dtype=np.float32)
    w3 = np.asarray(w3, dtype=np.float32)
    b3 = np.asarray(b3, dtype=np.float32)
    w2 = np.asarray(w2, dtype=np.float32)
    b2 = np.asarray(b2, dtype=np.float32)
    in_maps = []
    for e in range(E):
        in_maps.append(
            {
                "xt": np.ascontiguousarray(x[e].T),  # [D, T]
                "w1t": np.ascontiguousarray(w1[e].T),  # [D, F]
                "w3t": np.ascontiguousarray(w3[e].T),  # [D, F]
                "w2t": np.ascontiguousarray(w2[e].T),  # [F, D]
                "b1": b1[e],
                "b3": b3[e],
                "b2r": np.ascontiguousarray(
                    np.broadcast_to(b2[e], (P, D))
                ),  # [P, D]
            }
        )
    return in_maps


def run(x, w1, b1, w3, b3, w2, b2, **spmd_kwargs):
    nc = build()
    in_maps = make_in_maps(x, w1, b1, w3, b3, w2, b2)
    res = run_bass_kernel_spmd(nc, in_maps, core_ids=list(range(E)), **spmd_kwargs)
    out = np.stack([res.results[e]["out"] for e in range(E)], axis=0)
    return out, res


def kernel(x, w1, b1, w3, b3, w2, b2):
    out, _ = run(x, w1, b1, w3, b3, w2, b2)
    return out
